# revision 21
# baseline (speedup 1.0000x reference)
"""Trainium2 Bass kernel for XCiT-style channel ("cross-covariance") attention.

Reference computation (per batch element b):
    qkv  = x @ w_qkv.T                    # [N, 3C]
    q,k,v -> [H, DH, N] (channel-major)
    q,k  l2-normalized along N (tokens)
    attn = softmax((q @ k^T) * temp)      # [H, DH, DH]
    out  = (attn @ v) -> [N, C] @ w_proj.T

Shapes: B=8, N=4096, C=512, H=8, DH=64.

Strategy: data-parallel over batch across the 8 NeuronCores (one batch
element per core, weights replicated, no collectives). All matmuls in
bf16 (fp32 accumulate in PSUM); elementwise/softmax math in fp32.

Per-core dataflow:
  Phase W: cast + PE-transpose w_qkv -> w_qkvT, w_proj -> w_projT (bf16).
  Phase A: per 512-token chunk: DMA x tiles, cast bf16, PE-transpose -> xT;
           q,k = (xT)^T @ w_qkvT[q|k]  (token-major, stored bf16)
           v   = (w_vT)^T @ xT         (channel-major, stored bf16)
  Phase B: token-norms of q,k via ones-matmul over q^2,k^2; per-head-pair
           Gram matmuls; fold temp/||q||, 1/||k|| into the 64x64 Gram;
           softmax along the free axis; PE-transpose block-diagonal
           attn -> attnT (bf16).
  Phase C: per chunk: out_att = attnT.T @ v (block-diag, one MM per pair),
           y = out_att.T @ w_projT (token-major) -> DMA out.
"""

import numpy as np

import concourse.bass as bass
import concourse.bacc as bacc
import concourse.mybir as mybir
import concourse.tile as tile
from concourse.masks import make_identity

F32 = mybir.dt.float32
BF16 = mybir.dt.bfloat16

N_TOK = 4096
C = 512
H = 8
DH = 64
P = 128
KT = C // P            # 4 c_in tiles
NT = N_TOK // P        # 32 token tiles
NCH = N_TOK // 512     # 8 token chunks
TPC = 4                # token tiles per chunk
N_CORES = 8

# experiment knobs (timing builds only; kernel() uses defaults)
CFG = {"phases": "WABC", "copy_mode": "alt", "psqk_bufs": 2, "psv_bufs": 2, "xT_bufs": 2, "sq_engine": "dve", "norm_in_a": True, "hint": True, "dma_split": False}


def build_bass(loop_n=None):
    nc = bacc.Bacc()

    x_d = nc.declare_dram_parameter("x", [N_TOK, C], BF16, isOutput=False)
    wqkv_d = nc.declare_dram_parameter("w_qkv", [3 * C, C], BF16, isOutput=False)
    wproj_d = nc.declare_dram_parameter("w_proj", [C, C], BF16, isOutput=False)
    temp_d = nc.declare_dram_parameter("temperature", [H, 1, 1], F32, isOutput=False)
    out_d = nc.declare_dram_parameter("out", [N_TOK, C], F32, isOutput=True)

    with tile.TileContext(nc) as tc:
        with tc.tile_pool(name="persist", bufs=1) as persist:
            ident = persist.tile([P, P], BF16, tag="ident")
            make_identity(nc, ident[:])

            ones_b = persist.tile([P, 1], BF16, tag="ones_b")
            nc.gpsimd.memset(ones_b[:], 1.0)

            # Persistent SBUF tensors
            wqkvT = persist.tile([P, KT, 3 * C], BF16, tag="wqkvT")
            wprojT = persist.tile([P, KT, C], BF16, tag="wprojT")
            q_sb = persist.tile([P, NT, C], BF16, tag="q_sb")
            k_sb = persist.tile([P, NT, C], BF16, tag="k_sb")
            v_sb = persist.tile([P, KT, N_TOK], BF16, tag="v_sb")
            weffT = persist.tile([P, KT, C], BF16, tag="weffT")
            rq_col = persist.tile([P, KT], F32, tag="rq_col")
            rk_bcast = persist.tile([P, C], F32, tag="rk_bcast")
            trow = persist.tile([1, C], F32, tag="trow")
            t8 = persist.tile([1, H], F32, tag="t8")

            # temperature -> [1, 8] -> broadcast to [1, 512] (c = h*64 + d)
            nc.sync.dma_start(
                out=t8[:], in_=temp_d.rearrange("h a b -> (a b) h")
            )
            nc.vector.tensor_copy(
                out=trow[0:1, :].rearrange("p (h d) -> p h d", d=DH),
                in_=t8[0:1, :].unsqueeze(-1).broadcast_to((1, H, DH)),
            )

            copy_flip = [0]

            def copy_out(dst_ap, src_ap):
                """PSUM->SBUF evacuation, alternating DVE / ACT."""
                mode = CFG.get("copy_mode", "alt")
                use_dve = mode == "dve" or (mode == "alt" and copy_flip[0] % 2 == 0)
                if use_dve:
                    nc.vector.tensor_copy(out=dst_ap, in_=src_ap)
                else:
                    nc.scalar.copy(out=dst_ap, in_=src_ap)
                copy_flip[0] += 1

            def phases():
                _emit(nc, tc, persist, copy_out, locals_d)

            locals_d = dict(
                ident=ident, ones_b=ones_b, wqkvT=wqkvT, wprojT=wprojT,
                q_sb=q_sb, k_sb=k_sb, v_sb=v_sb, weffT=weffT, rq_col=rq_col,
                rk_bcast=rk_bcast, trow=trow,
                x_d=x_d, wqkv_d=wqkv_d, wproj_d=wproj_d, out_d=out_d,
            )
            if loop_n is None:
                phases()
            else:
                hint = tuple(nc.engines.keys()) if CFG.get("hint") else ()
                with tc.For_i(0, loop_n, 1, hint_engines=hint):
                    phases()

    nc.compile()
    return nc


def _emit(nc, tc, persist, copy_out, L):
    ident, ones_b, wqkvT, wprojT = L["ident"], L["ones_b"], L["wqkvT"], L["wprojT"]
    q_sb, k_sb, v_sb, weffT = L["q_sb"], L["k_sb"], L["v_sb"], L["weffT"]
    rq_col, rk_bcast, trow = L["rq_col"], L["rk_bcast"], L["trow"]
    x_d, wqkv_d, wproj_d, out_d = L["x_d"], L["wqkv_d"], L["wproj_d"], L["out_d"]
    phases_on = CFG.get("phases", "WABC")

    psn = tc.alloc_tile_pool(name="psn", bufs=1, space="PSUM")
    norm_q = psn.tile([1, C], F32, tag="norm_q")
    norm_k = psn.tile([1, C], F32, tag="norm_k")
    sqp = tc.alloc_tile_pool(name="sqp", bufs=3)

    def square(dst, src_ap):
        if CFG.get("sq_engine") == "act":
            nc.scalar.activation(
                dst, src_ap, mybir.ActivationFunctionType.Square
            )
        else:
            nc.vector.tensor_mul(out=dst, in0=src_ap, in1=src_ap)

    def norm_mms(g):
        q2 = sqp.tile([P, C], BF16, tag="q2", name="q2")
        square(q2[:], q_sb[:, g, :])
        nc.tensor.matmul(
            norm_q[:], ones_b[:], q2[:],
            start=(g == 0), stop=(g == NT - 1),
        )
        k2 = sqp.tile([P, C], BF16, tag="k2", name="k2")
        square(k2[:], k_sb[:, g, :])
        nc.tensor.matmul(
            norm_k[:], ones_b[:], k2[:],
            start=(g == 0), stop=(g == NT - 1),
        )

    if "W" in phases_on:
        if True:
            # ---- Phase W: XBAR-transpose weights from DRAM ----
            for k in range(KT):
                for c3 in range(3):  # w_qkv in three 512-row groups
                    nc.sync.dma_start(
                        out=wqkvT[:, k, c3 * C:(c3 + 1) * C],
                        in_=wqkv_d[c3 * C:(c3 + 1) * C, k * P:(k + 1) * P],
                        transpose=True,
                    )
                nc.sync.dma_start(
                    out=wprojT[:, k, :],
                    in_=wproj_d[:, k * P:(k + 1) * P],
                    transpose=True,
                )

    if "A" in phases_on:
        if True:
            # ---- Phase A: xT, q, k (token-major) and v (channel-major) ----
            with (
                tc.tile_pool(name="xTp", bufs=CFG["xT_bufs"]) as xTp,
                tc.tile_pool(name="psqk", bufs=CFG["psqk_bufs"], space="PSUM") as psqk,
                tc.tile_pool(name="psv", bufs=CFG["psv_bufs"], space="PSUM") as psv,
            ):
                for ch in range(NCH):
                    xT = xTp.tile([P, KT, 512], BF16, tag="xT")
                    xdma = nc.scalar if CFG.get("dma_split") else nc.sync
                    for k in range(KT):
                        xdma.dma_start(
                            out=xT[:, k, :],
                            in_=x_d[ch * 512:(ch + 1) * 512, k * P:(k + 1) * P],
                            transpose=True,
                        )
                    # q, k (token-major): lhsT = xT tile, rhs = w_qkvT cols
                    for t in range(TPC):
                        g = ch * TPC + t
                        for idx, dst in ((0, q_sb), (1, k_sb)):
                            ps = psqk.tile([P, 512], F32, tag="psqk")
                            for k in range(KT):
                                nc.tensor.matmul(
                                    ps[:],
                                    xT[:, k, t * P:(t + 1) * P],
                                    wqkvT[:, k, idx * C:(idx + 1) * C],
                                    start=(k == 0),
                                    stop=(k == KT - 1),
                                )
                            copy_out(dst[:, g, :], ps[:])
                        if CFG.get("norm_in_a"):
                            norm_mms(g)
                    # v (channel-major): lhsT = w_vT tile, rhs = xT chunk
                    for j in range(KT):
                        ps = psv.tile([P, 512], F32, tag="psv")
                        for k in range(KT):
                            nc.tensor.matmul(
                                ps[:],
                                wqkvT[:, k, 2 * C + j * P:2 * C + (j + 1) * P],
                                xT[:, k, :],
                                start=(k == 0),
                                stop=(k == KT - 1),
                            )
                        copy_out(v_sb[:, j, ch * 512:(ch + 1) * 512], ps[:])

    if "B" in phases_on:
        if True:
            # ---- Phase B: norms, Grams, softmax, attnT ----
            with (
                tc.tile_pool(name="smp", bufs=2) as smp,
                tc.tile_pool(name="psg", bufs=1, space="PSUM") as psg,
                tc.tile_pool(name="psat", bufs=2, space="PSUM") as psat,
            ):
                gram = [
                    psg.tile([P, P], F32, tag=f"gram{p}", name=f"gram{p}")
                    for p in range(4)
                ]

                for g in range(NT):
                    if not CFG.get("norm_in_a"):
                        norm_mms(g)
                    for p in range(4):
                        nc.tensor.matmul(
                            gram[p][:],
                            q_sb[:, g, p * P:(p + 1) * P],
                            k_sb[:, g, p * P:(p + 1) * P],
                            start=(g == 0), stop=(g == NT - 1),
                        )

                # rq = temp / ||q||, rk = 1 / ||k||   (rows [1, 512])
                rq_row = smp.tile([1, C], F32, tag="rq_row")
                rk_row = smp.tile([1, C], F32, tag="rk_row")
                sq_t = smp.tile([1, C], F32, tag="sq_t")
                nc.scalar.activation(
                    sq_t[:], norm_q[:], mybir.ActivationFunctionType.Sqrt
                )
                nc.vector.reciprocal(rq_row[:], sq_t[:])
                nc.vector.tensor_mul(out=rq_row[:], in0=rq_row[:], in1=trow[:])
                sk_t = smp.tile([1, C], F32, tag="sk_t")
                nc.scalar.activation(
                    sk_t[:], norm_k[:], mybir.ActivationFunctionType.Sqrt
                )
                nc.vector.reciprocal(rk_row[:], sk_t[:])

                # rq as per-partition column tiles [128, 4]; rk broadcast rows
                for j in range(KT):
                    nc.sync.dma_start(
                        out=rq_col[:, j:j + 1],
                        in_=rq_row[0:1, j * P:(j + 1) * P],
                    )
                nc.sync.dma_start(
                    out=rk_bcast[:],
                    in_=rk_row[0:1, :].unsqueeze(1).broadcast_to((1, P, C)),
                )

                # softmax per head pair -> block-diagonal attn (bf16)
                # then w_eff^T[he, c_out] = sum_d attn[d, e] wprojT[hd, c_out]
                for p in range(4):
                    abd = smp.tile([P, P], BF16, tag="abd")
                    nc.gpsimd.memset(abd[:], 0.0)
                    tmp = smp.tile([P, P], F32, tag="sm_tmp")
                    nc.vector.tensor_scalar_mul(
                        tmp[:], gram[p][:], rq_col[:, p:p + 1]
                    )
                    nc.vector.tensor_mul(
                        out=tmp[:], in0=tmp[:],
                        in1=rk_bcast[:, p * P:(p + 1) * P],
                    )
                    et = smp.tile([P, P], F32, tag="sm_e")
                    nc.scalar.activation(
                        et[:], tmp[:], mybir.ActivationFunctionType.Exp
                    )
                    ssum = smp.tile([P, 1], F32, tag="sm_s")
                    srcp = smp.tile([P, 1], F32, tag="sm_r")
                    for hh in range(2):
                        sl = slice(hh * DH, (hh + 1) * DH)
                        nc.vector.reduce_sum(
                            ssum[sl, :], et[sl, sl],
                            axis=mybir.AxisListType.X,
                        )
                        nc.vector.reciprocal(srcp[sl, :], ssum[sl, :])
                        nc.vector.tensor_scalar_mul(
                            abd[sl, sl], et[sl, sl], srcp[sl, 0:1]
                        )
                    ps = psat.tile([P, 512], F32, tag="psat")
                    nc.tensor.matmul(
                        ps[:], abd[:], wprojT[:, p, :], start=True, stop=True
                    )
                    copy_out(weffT[:, p, :], ps[:])

    if "C" in phases_on:
        if True:
            # ---- Phase C: y[tok, c_out] = sum_he v[he, tok] * weffT[he, c_out]
            with (
                tc.tile_pool(name="yp", bufs=3) as yp,
                tc.tile_pool(name="psy", bufs=2, space="PSUM") as psy,
            ):
                for ch in range(NCH):
                    yc = yp.tile([P, TPC, C], F32, tag="yc")
                    for t in range(TPC):
                        g = ch * TPC + t
                        ps = psy.tile([P, 512], F32, tag="psy")
                        for j in range(KT):
                            nc.tensor.matmul(
                                ps[:],
                                v_sb[:, j, g * P:(g + 1) * P],
                                weffT[:, j, :],
                                start=(j == 0), stop=(j == KT - 1),
                            )
                        copy_out(yc[:, t, :], ps[:])
                    nc.sync.dma_start(
                        out=out_d[ch * 512:(ch + 1) * 512, :].rearrange(
                            "(t p) c -> p t c", p=P
                        ),
                        in_=yc[:],
                    )

    psn.release()
    sqp.release()


_NC_CACHE = None


def _get_nc():
    global _NC_CACHE
    if _NC_CACHE is None:
        _NC_CACHE = build_bass()
    return _NC_CACHE


def make_in_maps(x, w_qkv, w_proj, temperature):
    """Shard inputs for the 8 cores; x/weights pre-cast to bf16 on host
    (the kernel computes its matmuls in bf16 either way)."""
    import ml_dtypes

    bf = ml_dtypes.bfloat16
    x = np.ascontiguousarray(np.asarray(x, dtype=np.float32).astype(bf))
    w_qkv = np.ascontiguousarray(np.asarray(w_qkv, dtype=np.float32).astype(bf))
    w_proj = np.ascontiguousarray(np.asarray(w_proj, dtype=np.float32).astype(bf))
    temperature = np.ascontiguousarray(np.asarray(temperature, dtype=np.float32))
    return [
        {
            "x": x[b],
            "w_qkv": w_qkv,
            "w_proj": w_proj,
            "temperature": temperature,
        }
        for b in range(N_CORES)
    ]


def kernel(**inputs) -> np.ndarray:
    from concourse.bass_utils import run_bass_kernel_spmd

    nc = _get_nc()
    in_maps = make_in_maps(
        inputs["x"], inputs["w_qkv"], inputs["w_proj"], inputs["temperature"]
    )
    res = run_bass_kernel_spmd(nc, in_maps, core_ids=list(range(N_CORES)))
    return np.stack([res.results[b]["out"] for b in range(N_CORES)], axis=0)


# revision 23
# speedup vs baseline: 1.1098x; 1.1098x over previous
"""Trainium2 Bass kernel for XCiT-style channel ("cross-covariance") attention.

Reference computation (per batch element b):
    qkv  = x @ w_qkv.T                    # [N, 3C]
    q,k,v -> [H, DH, N] (channel-major)
    q,k  l2-normalized along N (tokens)
    attn = softmax((q @ k^T) * temp)      # [H, DH, DH]
    out  = (attn @ v) -> [N, C] @ w_proj.T

Shapes: B=8, N=4096, C=512, H=8, DH=64.

Strategy: data-parallel over batch across the 8 NeuronCores (one batch
element per core, weights replicated, no collectives). All matmuls in
bf16 (fp32 accumulate in PSUM); elementwise/softmax math in fp32.

Inputs x / w_qkv / w_proj are pre-cast to bf16 on the host (the kernel
computes its matmuls in bf16 regardless), which enables DMA-XBAR
transposed loads straight from DRAM.

Per-core dataflow:
  Phase W: XBAR-transposed DMA loads w_qkv^T, w_proj^T (bf16, no PE work).
  Phase A: per 512-token chunk: XBAR-transposed DMA load of x^T;
           q,k = (xT)^T @ w_qkvT[q|k]  (token-major, stored bf16)
           v   = (w_vT)^T @ xT         (channel-major, stored bf16)
           plus per-tile token-norm matmuls (ones^T @ q^2 / k^2).
  Phase B: per-head-pair Gram matmuls (two heads block-packed in 128x128);
           fold temp/||q||, 1/||k|| scalings into the 64x64 Gram; softmax
           along the free axis into block-diagonal attn (bf16); then fuse
           attn with the output projection:
             w_eff^T[he, c_out] = sum_d attn_h[d, e] * w_projT[hd, c_out].
  Phase C: y[tok, c_out] = sum_he v[he, tok] * w_effT[he, c_out]
           (token-major) -> contiguous DMA out. The attn@v GEMM and the
           output projection collapse into this single pass over v.
"""

import numpy as np

import concourse.bacc as bacc
import concourse.mybir as mybir
import concourse.tile as tile

F32 = mybir.dt.float32
BF16 = mybir.dt.bfloat16

N_TOK = 4096
C = 512
H = 8
DH = 64
P = 128
KT = C // P            # 4 c_in tiles
NT = N_TOK // P        # 32 token tiles
NCH = N_TOK // 512     # 8 token chunks
TPC = 4                # token tiles per chunk
N_CORES = 8

# experiment knobs (timing builds only; kernel() uses defaults)
CFG = {"phases": "WABC", "copy_mode": "alt", "psqk_bufs": 2, "psv_bufs": 2, "xT_bufs": 2, "sq_engine": "dve", "norm_in_a": True, "hint": True, "dma_split": False}


def build_bass(loop_n=None):
    nc = bacc.Bacc()

    x_d = nc.declare_dram_parameter("x", [N_TOK, C], BF16, isOutput=False)
    wqkv_d = nc.declare_dram_parameter("w_qkv", [3 * C, C], BF16, isOutput=False)
    wproj_d = nc.declare_dram_parameter("w_proj", [C, C], BF16, isOutput=False)
    temp_d = nc.declare_dram_parameter("temperature", [H, 1, 1], F32, isOutput=False)
    out_d = nc.declare_dram_parameter("out", [N_TOK, C], F32, isOutput=True)

    with tile.TileContext(nc) as tc:
        with tc.tile_pool(name="persist", bufs=1) as persist:
            ones_b = persist.tile([P, 1], BF16, tag="ones_b")
            nc.gpsimd.memset(ones_b[:], 1.0)

            # Persistent SBUF tensors
            wqkvT = persist.tile([P, KT, 3 * C], BF16, tag="wqkvT")
            wprojT = persist.tile([P, KT, C], BF16, tag="wprojT")
            q_sb = persist.tile([P, NT, C], BF16, tag="q_sb")
            k_sb = persist.tile([P, NT, C], BF16, tag="k_sb")
            v_sb = persist.tile([P, KT, N_TOK], BF16, tag="v_sb")
            weffT = persist.tile([P, KT, C], BF16, tag="weffT")
            rq_col = persist.tile([P, KT], F32, tag="rq_col")
            rk_bcast = persist.tile([P, C], F32, tag="rk_bcast")
            trow = persist.tile([1, C], F32, tag="trow")
            t8 = persist.tile([1, H], F32, tag="t8")

            # temperature -> [1, 8] -> broadcast to [1, 512] (c = h*64 + d)
            nc.sync.dma_start(
                out=t8[:], in_=temp_d.rearrange("h a b -> (a b) h")
            )
            nc.vector.tensor_copy(
                out=trow[0:1, :].rearrange("p (h d) -> p h d", d=DH),
                in_=t8[0:1, :].unsqueeze(-1).broadcast_to((1, H, DH)),
            )

            copy_flip = [0]

            def copy_out(dst_ap, src_ap):
                """PSUM->SBUF evacuation, alternating DVE / ACT."""
                mode = CFG.get("copy_mode", "alt")
                use_dve = mode == "dve" or (mode == "alt" and copy_flip[0] % 2 == 0)
                if use_dve:
                    nc.vector.tensor_copy(out=dst_ap, in_=src_ap)
                else:
                    nc.scalar.copy(out=dst_ap, in_=src_ap)
                copy_flip[0] += 1

            def phases():
                _emit(nc, tc, persist, copy_out, locals_d)

            locals_d = dict(
                ones_b=ones_b, wqkvT=wqkvT, wprojT=wprojT,
                q_sb=q_sb, k_sb=k_sb, v_sb=v_sb, weffT=weffT, rq_col=rq_col,
                rk_bcast=rk_bcast, trow=trow,
                x_d=x_d, wqkv_d=wqkv_d, wproj_d=wproj_d, out_d=out_d,
            )
            if loop_n is None:
                phases()
            else:
                hint = tuple(nc.engines.keys()) if CFG.get("hint") else ()
                with tc.For_i(0, loop_n, 1, hint_engines=hint):
                    phases()

    nc.compile()
    return nc


def _emit(nc, tc, persist, copy_out, L):
    ones_b, wqkvT, wprojT = L["ones_b"], L["wqkvT"], L["wprojT"]
    q_sb, k_sb, v_sb, weffT = L["q_sb"], L["k_sb"], L["v_sb"], L["weffT"]
    rq_col, rk_bcast, trow = L["rq_col"], L["rk_bcast"], L["trow"]
    x_d, wqkv_d, wproj_d, out_d = L["x_d"], L["wqkv_d"], L["wproj_d"], L["out_d"]
    phases_on = CFG.get("phases", "WABC")

    psn = tc.alloc_tile_pool(name="psn", bufs=1, space="PSUM")
    norm_q = psn.tile([1, C], F32, tag="norm_q")
    norm_k = psn.tile([1, C], F32, tag="norm_k")
    sqp = tc.alloc_tile_pool(name="sqp", bufs=3)

    def square(dst, src_ap):
        if CFG.get("sq_engine") == "act":
            nc.scalar.activation(
                dst, src_ap, mybir.ActivationFunctionType.Square
            )
        else:
            nc.vector.tensor_mul(out=dst, in0=src_ap, in1=src_ap)

    def norm_mms(g):
        q2 = sqp.tile([P, C], BF16, tag="q2", name="q2")
        square(q2[:], q_sb[:, g, :])
        nc.tensor.matmul(
            norm_q[:], ones_b[:], q2[:],
            start=(g == 0), stop=(g == NT - 1),
        )
        k2 = sqp.tile([P, C], BF16, tag="k2", name="k2")
        square(k2[:], k_sb[:, g, :])
        nc.tensor.matmul(
            norm_k[:], ones_b[:], k2[:],
            start=(g == 0), stop=(g == NT - 1),
        )

    if "W" in phases_on:
        if True:
            # ---- Phase W: XBAR-transpose weights from DRAM ----
            for k in range(KT):
                for c3 in range(3):  # w_qkv in three 512-row groups
                    nc.sync.dma_start(
                        out=wqkvT[:, k, c3 * C:(c3 + 1) * C],
                        in_=wqkv_d[c3 * C:(c3 + 1) * C, k * P:(k + 1) * P],
                        transpose=True,
                    )
                nc.sync.dma_start(
                    out=wprojT[:, k, :],
                    in_=wproj_d[:, k * P:(k + 1) * P],
                    transpose=True,
                )

    if "A" in phases_on:
        if True:
            # ---- Phase A: xT, q, k (token-major) and v (channel-major) ----
            with (
                tc.tile_pool(name="xTp", bufs=CFG["xT_bufs"]) as xTp,
                tc.tile_pool(name="psqk", bufs=CFG["psqk_bufs"], space="PSUM") as psqk,
                tc.tile_pool(name="psv", bufs=CFG["psv_bufs"], space="PSUM") as psv,
            ):
                for ch in range(NCH):
                    xT = xTp.tile([P, KT, 512], BF16, tag="xT")
                    xdma = nc.scalar if CFG.get("dma_split") else nc.sync
                    for k in range(KT):
                        xdma.dma_start(
                            out=xT[:, k, :],
                            in_=x_d[ch * 512:(ch + 1) * 512, k * P:(k + 1) * P],
                            transpose=True,
                        )
                    # q, k (token-major): lhsT = xT tile, rhs = w_qkvT cols
                    for t in range(TPC):
                        g = ch * TPC + t
                        for idx, dst in ((0, q_sb), (1, k_sb)):
                            ps = psqk.tile([P, 512], F32, tag="psqk")
                            for k in range(KT):
                                nc.tensor.matmul(
                                    ps[:],
                                    xT[:, k, t * P:(t + 1) * P],
                                    wqkvT[:, k, idx * C:(idx + 1) * C],
                                    start=(k == 0),
                                    stop=(k == KT - 1),
                                )
                            copy_out(dst[:, g, :], ps[:])
                        if CFG.get("norm_in_a"):
                            norm_mms(g)
                    # v (channel-major): lhsT = w_vT tile, rhs = xT chunk
                    for j in range(KT):
                        ps = psv.tile([P, 512], F32, tag="psv")
                        for k in range(KT):
                            nc.tensor.matmul(
                                ps[:],
                                wqkvT[:, k, 2 * C + j * P:2 * C + (j + 1) * P],
                                xT[:, k, :],
                                start=(k == 0),
                                stop=(k == KT - 1),
                            )
                        copy_out(v_sb[:, j, ch * 512:(ch + 1) * 512], ps[:])

    if "B" in phases_on:
        if True:
            # ---- Phase B: norms, Grams, softmax, attnT ----
            with (
                tc.tile_pool(name="smp", bufs=2) as smp,
                tc.tile_pool(name="psg", bufs=1, space="PSUM") as psg,
                tc.tile_pool(name="psat", bufs=2, space="PSUM") as psat,
            ):
                gram = [
                    psg.tile([P, P], F32, tag=f"gram{p}", name=f"gram{p}")
                    for p in range(4)
                ]

                for g in range(NT):
                    if not CFG.get("norm_in_a"):
                        norm_mms(g)
                    for p in range(4):
                        nc.tensor.matmul(
                            gram[p][:],
                            q_sb[:, g, p * P:(p + 1) * P],
                            k_sb[:, g, p * P:(p + 1) * P],
                            start=(g == 0), stop=(g == NT - 1),
                        )

                # rq = temp / ||q||, rk = 1 / ||k||   (rows [1, 512])
                rq_row = smp.tile([1, C], F32, tag="rq_row")
                rk_row = smp.tile([1, C], F32, tag="rk_row")
                sq_t = smp.tile([1, C], F32, tag="sq_t")
                nc.scalar.activation(
                    sq_t[:], norm_q[:], mybir.ActivationFunctionType.Sqrt
                )
                nc.vector.reciprocal(rq_row[:], sq_t[:])
                nc.vector.tensor_mul(out=rq_row[:], in0=rq_row[:], in1=trow[:])
                sk_t = smp.tile([1, C], F32, tag="sk_t")
                nc.scalar.activation(
                    sk_t[:], norm_k[:], mybir.ActivationFunctionType.Sqrt
                )
                nc.vector.reciprocal(rk_row[:], sk_t[:])

                # rq as per-partition column tiles [128, 4]; rk broadcast rows
                for j in range(KT):
                    nc.sync.dma_start(
                        out=rq_col[:, j:j + 1],
                        in_=rq_row[0:1, j * P:(j + 1) * P],
                    )
                nc.sync.dma_start(
                    out=rk_bcast[:],
                    in_=rk_row[0:1, :].unsqueeze(1).broadcast_to((1, P, C)),
                )

                # softmax per head pair -> block-diagonal attn (bf16)
                # then w_eff^T[he, c_out] = sum_d attn[d, e] wprojT[hd, c_out]
                for p in range(4):
                    abd = smp.tile([P, P], BF16, tag="abd")
                    nc.gpsimd.memset(abd[:], 0.0)
                    tmp = smp.tile([P, P], F32, tag="sm_tmp")
                    nc.vector.tensor_scalar_mul(
                        tmp[:], gram[p][:], rq_col[:, p:p + 1]
                    )
                    nc.vector.tensor_mul(
                        out=tmp[:], in0=tmp[:],
                        in1=rk_bcast[:, p * P:(p + 1) * P],
                    )
                    et = smp.tile([P, P], F32, tag="sm_e")
                    nc.scalar.activation(
                        et[:], tmp[:], mybir.ActivationFunctionType.Exp
                    )
                    ssum = smp.tile([P, 1], F32, tag="sm_s")
                    srcp = smp.tile([P, 1], F32, tag="sm_r")
                    for hh in range(2):
                        sl = slice(hh * DH, (hh + 1) * DH)
                        nc.vector.reduce_sum(
                            ssum[sl, :], et[sl, sl],
                            axis=mybir.AxisListType.X,
                        )
                        nc.vector.reciprocal(srcp[sl, :], ssum[sl, :])
                        nc.vector.tensor_scalar_mul(
                            abd[sl, sl], et[sl, sl], srcp[sl, 0:1]
                        )
                    ps = psat.tile([P, 512], F32, tag="psat")
                    nc.tensor.matmul(
                        ps[:], abd[:], wprojT[:, p, :], start=True, stop=True
                    )
                    copy_out(weffT[:, p, :], ps[:])

    if "C" in phases_on:
        if True:
            # ---- Phase C: y[tok, c_out] = sum_he v[he, tok] * weffT[he, c_out]
            with (
                tc.tile_pool(name="yp", bufs=3) as yp,
                tc.tile_pool(name="psy", bufs=2, space="PSUM") as psy,
            ):
                for ch in range(NCH):
                    yc = yp.tile([P, TPC, C], F32, tag="yc")
                    for t in range(TPC):
                        g = ch * TPC + t
                        ps = psy.tile([P, 512], F32, tag="psy")
                        for j in range(KT):
                            nc.tensor.matmul(
                                ps[:],
                                v_sb[:, j, g * P:(g + 1) * P],
                                weffT[:, j, :],
                                start=(j == 0), stop=(j == KT - 1),
                            )
                        copy_out(yc[:, t, :], ps[:])
                    nc.sync.dma_start(
                        out=out_d[ch * 512:(ch + 1) * 512, :].rearrange(
                            "(t p) c -> p t c", p=P
                        ),
                        in_=yc[:],
                    )

    psn.release()
    sqp.release()


_NC_CACHE = None


def _get_nc():
    global _NC_CACHE
    if _NC_CACHE is None:
        _NC_CACHE = build_bass()
    return _NC_CACHE


def make_in_maps(x, w_qkv, w_proj, temperature):
    """Shard inputs for the 8 cores; x/weights pre-cast to bf16 on host
    (the kernel computes its matmuls in bf16 either way)."""
    import ml_dtypes

    bf = ml_dtypes.bfloat16
    x = np.ascontiguousarray(np.asarray(x, dtype=np.float32).astype(bf))
    w_qkv = np.ascontiguousarray(np.asarray(w_qkv, dtype=np.float32).astype(bf))
    w_proj = np.ascontiguousarray(np.asarray(w_proj, dtype=np.float32).astype(bf))
    temperature = np.ascontiguousarray(np.asarray(temperature, dtype=np.float32))
    return [
        {
            "x": x[b],
            "w_qkv": w_qkv,
            "w_proj": w_proj,
            "temperature": temperature,
        }
        for b in range(N_CORES)
    ]


def kernel(**inputs) -> np.ndarray:
    from concourse.bass_utils import run_bass_kernel_spmd

    nc = _get_nc()
    in_maps = make_in_maps(
        inputs["x"], inputs["w_qkv"], inputs["w_proj"], inputs["temperature"]
    )
    res = run_bass_kernel_spmd(nc, in_maps, core_ids=list(range(N_CORES)))
    return np.stack([res.results[b]["out"] for b in range(N_CORES)], axis=0)
